# revision 2
# baseline (speedup 1.0000x reference)
"""CNN+LSTM seq2seq kernel for trn2, 8-core data parallel — v2.

Key layout change vs v1: gates live in PSUM as [128, 1024] where partition
p = b + 64*hh (hh = hidden half) via column-tiled matmuls (two concurrent
col-groups on the PE array). Halves both PE streaming columns and every
elementwise op's free dim. Gate column order [g f i o] x 256; bank0 =
{g,f}, bank1 = {i,o} with i/o as separate N=256 groups so sigma(i) can
issue before the o matmuls finish.

Sharding: batch 512 split across 8 cores; weights replicated.
"""

import numpy as np

import concourse.bass as bass
import concourse.mybir as mybir
import concourse.tile as tile_mod
from concourse import bacc
from concourse.masks import make_identity

F32 = mybir.dt.float32
F32R = mybir.dt.float32r
BF16 = mybir.dt.bfloat16
AF = mybir.ActivationFunctionType

B = 64        # batch per core
S = 1024      # input seq len
CIN = 16
OC = 64       # conv out channels
KW = 5
T2 = 512      # encoder steps after pool
H = 512       # hidden
HH = 256      # per-col-group gate block width
OUT_STEPS = 64
NCORES = 8

# torch gate blocks (rows of W): 0=i 1=f 2=g 3=o. Column position X in the
# col-tiled layout: [g f i o].
X_OF_TORCH = {0: 2, 1: 1, 2: 0, 3: 3}
TORCH_OF_X = [2, 1, 0, 3]

PHASES = {}


def _mark(nc, name):
    PHASES[name] = int(nc.get_next_instruction_name().split("-")[-1])


def build_nc(variant="full"):
    skip_conv = "noconv" in variant
    skip_elem = "mm" in variant
    nc = bacc.Bacc(None, target_bir_lowering=False, debug=False)

    # ---------- DRAM I/O ----------
    x_d = nc.dram_tensor("x", [B, S, CIN], F32, kind="ExternalInput")
    dstart_d = nc.dram_tensor("decoder_start", [B, 1], F32, kind="ExternalInput")
    convw_d = nc.dram_tensor("conv_w", [OC, CIN, KW], F32, kind="ExternalInput")
    convb_d = nc.dram_tensor("conv_b", [OC], F32, kind="ExternalInput")
    encWih_d = nc.dram_tensor("enc_Wih", [4 * H, OC], F32, kind="ExternalInput")
    encWhh_d = nc.dram_tensor("enc_Whh", [4 * H, H], F32, kind="ExternalInput")
    encb_d = nc.dram_tensor("enc_b", [4 * H], F32, kind="ExternalInput")
    decWih_d = nc.dram_tensor("dec_Wih", [4 * H, 1], F32, kind="ExternalInput")
    decWhh_d = nc.dram_tensor("dec_Whh", [4 * H, H], F32, kind="ExternalInput")
    decb_d = nc.dram_tensor("dec_b", [4 * H], F32, kind="ExternalInput")
    headw_d = nc.dram_tensor("head_w", [1, H], F32, kind="ExternalInput")
    headb_d = nc.dram_tensor("head_b", [1], F32, kind="ExternalInput")
    out_d = nc.dram_tensor("out", [B, OUT_STEPS], F32, kind="ExternalOutput")

    def mmr(out, lhsT, rhs, start, stop):
        nc.tensor.matmul(out, lhsT=lhsT.bitcast(F32R), rhs=rhs.bitcast(F32R),
                         start=start, stop=stop)

    with tile_mod.TileContext(nc) as tc:
        with tc.tile_pool(name="dram", bufs=1, space="DRAM") as dramp:
            enc_x = dramp.tile([T2, B, OC], F32)

            with tc.tile_pool(name="const", bufs=1) as cn:
                identity = cn.tile([128, 128], F32)
                make_identity(nc, identity)
                id64 = identity[:64, :64]
                idbf = cn.tile([128, 128], BF16)
                nc.vector.tensor_copy(idbf, identity)

                # persistent weights (f32r stream operands)
                hW = [[cn.tile([128, 1024], BF16, name=f"hW{cg}_{kc}")
                       for kc in range(4)] for cg in range(2)]
                dhW = [[cn.tile([128, 1024], BF16, name=f"dhW{cg}_{kc}")
                        for kc in range(4)] for cg in range(2)]
                xW = [cn.tile([OC + 1, 1024], BF16, name=f"xW{cg}") for cg in range(2)]
                dxW = [cn.tile([2, 1024], BF16, name=f"dxW{cg}") for cg in range(2)]
                cwT = [cn.tile([CIN, OC], BF16, name=f"cwT{k}") for k in range(KW)]
                cb = cn.tile([OC, 1], F32)
                cb2 = cn.tile([128, 1], F32)
                hdT = cn.tile([128, 4], BF16)
                hdstage = cn.tile([128, 4], F32)
                hb = cn.tile([1, 1], F32)
                ones_row = cn.tile([1, B], F32)
                nc.vector.memset(ones_row, 1.0)
                zpad = cn.tile([CIN, 2], F32)
                nc.vector.memset(zpad, 0.0)

                # persistent state, all [128, 256] (p = b + 64*hh)
                c_st = cn.tile([128, HH], F32)
                tg = cn.tile([128, HH], F32)
                sf = cn.tile([128, HH], F32)
                si = cn.tile([128, HH], F32)
                so = cn.tile([128, HH], F32)
                t1 = cn.tile([128, HH], F32)
                t2 = cn.tile([128, HH], F32)
                tcell = cn.tile([128, HH], F32)
                h_st = cn.tile([128, HH], BF16)
                # h.T: hTT1 = chunks {h 0:128 | h 256:384}, hTT2 = {128:256 | 384:512}
                hTT1 = cn.tile([128, 128], BF16)
                hTT2 = cn.tile([128, 128], BF16)
                # kc -> (tile, col offset)
                hTsl = [(hTT1, 0), (hTT2, 0), (hTT1, 64), (hTT2, 64)]
                xsT = [cn.tile([OC + 1, B], BF16, name=f"xsT{j}") for j in range(4)]
                onestage = cn.tile([2, B], F32)
                nc.vector.memset(onestage, 1.0)
                for j in range(4):
                    nc.vector.tensor_copy(xsT[j][OC:OC + 1, :], onestage[0:1, :])
                onesB = cn.tile([1, B], BF16)
                nc.vector.tensor_copy(onesB, onestage[0:1, :])
                # decoder feedback folded into weights; step-0 correction operand
                augz = cn.tile([1, B], BF16)
                dstart_row = cn.tile([1, B], F32)
                beff = [cn.tile([1, 1024], BF16, name=f"beff{cg}") for cg in range(2)]
                dstage_cg = [cn.tile([2, 1024], F32, name=f"dstage{cg}") for cg in range(2)]
                dbrow = [cn.tile([1, 1024], F32, name=f"dbrow{cg}") for cg in range(2)]
                outF = cn.tile([B, OUT_STEPS], F32)

                # ---------- weight prep ----------
                with (
                    tc.tile_pool(name="wtmp", bufs=3) as wt,
                    tc.tile_pool(name="wps", bufs=3, space="PSUM") as wps,
                ):
                    def prep_whh(src_d, dst, outer=None):
                        for jb in range(16):
                            X = X_OF_TORCH[jb // 4]
                            cg = (jb % 4) // 2
                            col0 = X * 256 + (jb % 2) * 128
                            wtmp = wt.tile([128, H], F32, tag="wtmp")
                            nc.sync.dma_start(out=wtmp, in_=src_d[128 * jb:128 * (jb + 1), :])
                            for kc in range(4):
                                wtp = wps.tile([128, 128], F32, tag="wtp")
                                nc.tensor.transpose(wtp, wtmp[:, 128 * kc:128 * (kc + 1)], identity)
                                dst_ap = dst[cg][kc][:, col0:col0 + 128]
                                if outer is not None:
                                    nc.vector.tensor_add(
                                        dst_ap, wtp,
                                        outer[cg][kc][:, col0:col0 + 128])
                                elif kc % 2 == 0:
                                    nc.scalar.copy(dst_ap, wtp)
                                else:
                                    nc.vector.tensor_copy(dst_ap, wtp)

                    prep_whh(encWhh_d, hW)

                    # enc_Wih.T into xW rows 0..63
                    for jb in range(16):
                        X = X_OF_TORCH[jb // 4]
                        cg = (jb % 4) // 2
                        col0 = X * 256 + (jb % 2) * 128
                        wtmp2 = wt.tile([128, OC], F32, tag="wtmp2")
                        nc.sync.dma_start(out=wtmp2, in_=encWih_d[128 * jb:128 * (jb + 1), :])
                        wtp = wps.tile([128, 128], F32, tag="wtp")
                        nc.tensor.transpose(wtp[:OC, :128], wtmp2, identity)
                        nc.scalar.copy(xW[cg][0:OC, col0:col0 + 128], wtp[:OC, :128])
                    # biases / dec vectors via fp32 staging then engine round
                    for cg in range(2):
                        bstage = wt.tile([1, 1024], F32, tag="bstage", bufs=2)
                        dstage = dstage_cg[cg]
                        for X in range(4):
                            r0 = TORCH_OF_X[X] * 512 + cg * 256
                            nc.sync.dma_start(out=bstage[0:1, X * 256:(X + 1) * 256],
                                              in_=encb_d[None, r0:r0 + 256])
                            nc.sync.dma_start(out=dstage[0:1, X * 256:(X + 1) * 256],
                                              in_=decWih_d[r0:r0 + 256, :].rearrange("a b -> b a"))
                            nc.sync.dma_start(out=dstage[1:2, X * 256:(X + 1) * 256],
                                              in_=decb_d[None, r0:r0 + 256])
                            nc.sync.dma_start(out=dbrow[cg][0:1, X * 256:(X + 1) * 256],
                                              in_=decb_d[None, r0:r0 + 256])
                        nc.scalar.copy(xW[cg][OC:OC + 1, :], bstage)
                        nc.vector.tensor_copy(dxW[cg][:, :], dstage)
                    # outer(head_w, dec_Wih) staged in fp32; dhW = Whh.T + outer
                    hwrow = wt.tile([1, H], F32, tag="hwrow", bufs=1, name="hwrow")
                    nc.sync.dma_start(out=hwrow, in_=headw_d[:, :])
                    outer_sb = [[wt.tile([128, 1024], F32, tag=f"osb{cg}_{kc}", bufs=1,
                                          name=f"osb{cg}_{kc}")
                                 for kc in range(4)] for cg in range(2)]
                    for cg in range(2):
                        for kc in range(4):
                            for half in range(2):
                                ops = wps.tile([128, 512], F32, tag="ops", bufs=2)
                                nc.tensor.matmul(
                                    ops, lhsT=hwrow[0:1, 128 * kc:128 * (kc + 1)],
                                    rhs=dstage_cg[cg][0:1, 512 * half:512 * (half + 1)],
                                    start=True, stop=True)
                                nc.vector.tensor_copy(
                                    outer_sb[cg][kc][:, 512 * half:512 * (half + 1)], ops)
                    prep_whh(decWhh_d, dhW, outer=outer_sb)
                    # beff = dec_b + head_b * dec_Wih  (rank-0 via tiny matmuls)
                    nc.sync.dma_start(out=hb, in_=headb_d[:, None])
                    for cg in range(2):
                        for half in range(2):
                            bps = wps.tile([1, 512], F32, tag="bps", bufs=2)
                            nc.tensor.matmul(bps, lhsT=hb,
                                             rhs=dstage_cg[cg][0:1, 512 * half:512 * (half + 1)],
                                             start=True, stop=False)
                            nc.tensor.matmul(bps, lhsT=ones_row[:, 0:1],
                                             rhs=dbrow[cg][0:1, 512 * half:512 * (half + 1)],
                                             start=False, stop=True)
                            nc.scalar.copy(beff[cg][0:1, 512 * half:512 * (half + 1)], bps)
                    # conv weights: cwT[k][ic, oc] = conv_w[oc, ic, k]
                    cstage = wt.tile([CIN, KW * OC], F32, tag="cstage", bufs=1)
                    for k in range(KW):
                        nc.sync.dma_start(
                            out=cstage[:, OC * k:OC * (k + 1)],
                            in_=convw_d[:, :, k].rearrange("oc ic -> ic oc"),
                        )
                        nc.scalar.copy(cwT[k][:, :], cstage[:, OC * k:OC * (k + 1)])
                    nc.sync.dma_start(out=cb, in_=convb_d[:, None])
                    nc.scalar.copy(cb2[0:64, :], cb)
                    nc.scalar.copy(cb2[64:128, :], cb)
                    for kc in range(4):
                        nc.sync.dma_start(
                            out=hdstage[:, kc:kc + 1],
                            in_=headw_d[:, 128 * kc:128 * (kc + 1)].rearrange("a b -> b a"),
                        )
                    nc.vector.tensor_copy(hdT, hdstage)
                    nc.sync.dma_start(out=dstart_row, in_=dstart_d[:, :].rearrange("a b -> b a"))

                _mark(nc, "conv_start")
                # ---------- conv + pool -> enc_x (same scheme as v1) ----------
                with (
                    tc.tile_pool(name="conv", bufs=2) as cp,
                    tc.tile_pool(name="convps", bufs=2, space="PSUM") as cpp,
                ):
                    if not skip_conv:
                        wmp = cpp.tile([128, 128], F32, tag="xtp")
                        for w in range(24):
                            nc.tensor.matmul(wmp, lhsT=identity, rhs=identity,
                                             start=True, stop=True)
                    # batch pairs: b-pair (2j, 2j+1) on PE col-groups 0/1
                    for j in ([] if skip_conv else range(B // 2)):
                        xTbs = []
                        for bb in range(2):
                            b = 2 * j + bb
                            xTb = cp.tile([32, S + 4 + 4], BF16, tag=f"xTb{bb}", bufs=2)
                            nc.vector.tensor_copy(xTb[0:CIN, 0:2], zpad)
                            nc.vector.tensor_copy(xTb[0:CIN, 2 + S:2 + S + 2], zpad)
                            for half in range(2):
                                xb_raw = cp.tile([128, 128], F32, tag="xb_raw", bufs=4)
                                nc.sync.dma_start(
                                    out=xb_raw.rearrange("p (a c) -> p a c", c=32)[:, :, 0:CIN],
                                    in_=x_d[b].rearrange("(a p) c -> p a c", p=128)[:, 4 * half:4 * half + 4, :],
                                )
                                xtp = cpp.tile([128, 128], F32, tag="xtp")
                                nc.tensor.transpose(xtp, xb_raw, identity)
                                for a in range(4):
                                    blk = xtp[32 * a:32 * (a + 1), :]
                                    dst = xTb[:, 2 + 128 * (4 * half + a):2 + 128 * (4 * half + a + 1)]
                                    if a % 2 == 0:
                                        nc.scalar.copy(dst, blk)
                                    else:
                                        nc.vector.tensor_copy(dst, blk)
                            xTbs.append(xTb)
                        yb = cp.tile([128, S], F32, tag="yb")
                        for half in range(2):
                            cps = cpp.tile([128, 512], F32, tag="cps")
                            for k in range(KW):
                                for cg in range(2):
                                    nc.tensor.matmul(
                                        cps[64 * cg:64 * (cg + 1), :], lhsT=cwT[k],
                                        rhs=xTbs[cg][0:CIN, k + 512 * half:k + 512 * half + 512],
                                        start=k == 0, stop=k == KW - 1)
                            nc.scalar.activation(yb[:, 512 * half:512 * (half + 1)], cps,
                                                 AF.Relu, bias=cb2[:, 0:1])
                        pooled = cp.tile([128, T2], F32, tag="pooled")
                        yb_pairs = yb.rearrange("p (t two) -> p t two", two=2)
                        nc.vector.tensor_max(pooled, yb_pairs[:, :, 0], yb_pairs[:, :, 1])
                        poolT = cp.tile([128, 512], F32, tag="poolT")
                        for q in range(4):
                            ptp = cpp.tile([128, 128], F32, tag="ptp")
                            nc.tensor.transpose(ptp, pooled[:, 128 * q:128 * (q + 1)], identity)
                            if q % 2 == 0:
                                nc.scalar.copy(poolT[:, 128 * q:128 * (q + 1)], ptp)
                            else:
                                nc.vector.tensor_copy(poolT[:, 128 * q:128 * (q + 1)], ptp)
                        for q in range(4):
                            nc.sync.dma_start(
                                out=enc_x[128 * q:128 * (q + 1), 2 * j:2 * j + 2, :],
                                in_=poolT[:, 128 * q:128 * (q + 1)],
                            )

                _mark(nc, "enc_start")
                # ---------- encoder + decoder ----------
                with (
                    tc.tile_pool(name="xbp", bufs=4) as xbp,
                    tc.tile_pool(name="gp", bufs=1, space="PSUM") as gpl,
                    tc.tile_pool(name="hp", bufs=1, space="PSUM") as hpl,
                ):
                    gf = [gpl.tile([128, 512], F32, name="gfA"),
                          gpl.tile([128, 512], F32, name="gfB")]
                    # gi/go padded to a full bank; the col 256:512 region of gi
                    # doubles as the xps transpose target, of go as the head psum
                    gi = [gpl.tile([128, 512], F32, name="giA"),
                          gpl.tile([128, 512], F32, name="giB")]
                    go = [gpl.tile([128, 512], F32, name="goA"),
                          gpl.tile([128, 512], F32, name="goB")]
                    htp0 = hpl.tile([128, 128], BF16, name="htp0")
                    htp1 = hpl.tile([128, 128], BF16, name="htp1")

                    def dma_xb(t):
                        xb = xbp.tile([B, OC], F32, tag="xb")
                        nc.sync.dma_start(out=xb, in_=enc_x[t])
                        return xb

                    def x_mms(t):
                        """x+bias matmuls for step t (start=True on each region)."""
                        j = t % 2
                        lhs = xsT[t % 4]
                        for cg in range(2):
                            sl = slice(64 * cg, 64 * (cg + 1))
                            w = xW[cg]
                            nc.tensor.matmul(gf[j][sl, 0:512], lhsT=lhs, rhs=w[:, 0:512],
                                             start=True, stop=False)
                            nc.tensor.matmul(gi[j][sl, 0:256], lhsT=lhs, rhs=w[:, 512:768],
                                             start=True, stop=False)
                            nc.tensor.matmul(go[j][sl, 0:256], lhsT=lhs, rhs=w[:, 768:1024],
                                             start=True, stop=False)

                    def h_mms(t, W, dec=False):
                        """recurrent matmuls for step t. Encoder: x-MMs opened the
                        accumulation (start there), we close it (stop on kc3).
                        Decoder: we open it (start on kc0), aug-MMs close it."""
                        j = t % 2
                        # gf region {g,f}: N=512
                        for kc in (0, 2, 1, 3):
                            hsl, c0 = hTsl[kc]
                            lhs = hsl[:, c0:c0 + 64]
                            for cg in range(2):
                                nc.tensor.matmul(gf[j][64 * cg:64 * (cg + 1), 0:512],
                                                 lhsT=lhs, rhs=W[cg][kc][:, 0:512],
                                                 start=dec and kc == 0,
                                                 stop=(not dec) and kc == 3)
                        # i then o: N=256
                        for dst, lo in ((gi, 512), (go, 768)):
                            for kc in (0, 2, 1, 3):
                                hsl, c0 = hTsl[kc]
                                lhs = hsl[:, c0:c0 + 64]
                                for cg in range(2):
                                    nc.tensor.matmul(dst[j][64 * cg:64 * (cg + 1), 0:256],
                                                     lhsT=lhs, rhs=W[cg][kc][:, lo:lo + 256],
                                                     start=dec and kc == 0,
                                                     stop=(not dec) and kc == 3)

                    def cell(t, first):
                        """LSTM cell elementwise on gps[t%2] -> c_st, h_st, hTT."""
                        if skip_elem:
                            return
                        j = t % 2
                        sl0, sl1 = slice(0, 128), slice(128, 256)
                        nc.scalar.activation(tg, gf[j][:, 0:256], AF.Tanh)
                        if not first:
                            nc.scalar.activation(sf, gf[j][:, 256:512], AF.Sigmoid)
                            nc.vector.tensor_mul(t1, sf, c_st)
                        # column-halved cell chain: half 0 releases hTT1 (and the
                        # kc0/kc2 matmuls) while half 1 is still in flight
                        nc.scalar.activation(si[:, sl0], gi[j][:, 0:128], AF.Sigmoid)
                        nc.scalar.activation(si[:, sl1], gi[j][:, 128:256], AF.Sigmoid)
                        nc.scalar.activation(so[:, sl0], go[j][:, 0:128], AF.Sigmoid)
                        nc.scalar.activation(so[:, sl1], go[j][:, 128:256], AF.Sigmoid)
                        for sl in (sl0, sl1):
                            if first:
                                nc.vector.tensor_mul(c_st[:, sl], si[:, sl], tg[:, sl])
                            else:
                                nc.vector.tensor_mul(t2[:, sl], si[:, sl], tg[:, sl])
                                nc.vector.tensor_add(c_st[:, sl], t1[:, sl], t2[:, sl])
                        nc.scalar.activation(tcell[:, sl0], c_st[:, sl0], AF.Tanh)
                        # keep-warm: dummy matmuls gated on mid-tail products keep
                        # HAM at K=8/8 across the PE gap (dead go padding region)
                        nc.tensor.matmul(go[t % 2][0:64, 320:448],
                                         lhsT=c_st[:, 0:64], rhs=c_st[:, 0:128],
                                         start=True, stop=True)
                        nc.scalar.activation(tcell[:, sl1], c_st[:, sl1], AF.Tanh)
                        nc.vector.tensor_mul(h_st[:, sl0], so[:, sl0], tcell[:, sl0])
                        nc.tensor.transpose(htp0, h_st[:, sl0], idbf)
                        nc.vector.tensor_mul(h_st[:, sl1], so[:, sl1], tcell[:, sl1])
                        nc.vector.tensor_copy(hTT1, htp0)
                        nc.tensor.transpose(htp1, h_st[:, sl1], idbf)
                        nc.scalar.copy(hTT2, htp1)

                    # --- PE warm-up: ~10us of dense matmuls so HAM latches
                    # K=8/8 before the steady loop (whose <3.4us idle gaps
                    # then never re-throttle it) ---
                    for w in range(24):
                        nc.tensor.matmul(gf[0][0:64, 0:512],
                                         lhsT=hW[0][0][:, 0:64],
                                         rhs=hW[0][1][:, 0:512],
                                         start=True, stop=True)
                    # --- encoder prologue: xb/xsT for t=0,1; x-MMs(0) ---
                    xbt = {}
                    for j in range(2):
                        xbt[j] = dma_xb(j)
                        nc.tensor.transpose(gi[j][0:64, 256:320], xbt[j], id64)
                        nc.scalar.copy(xsT[j][0:OC, :], gi[j][0:64, 256:320])
                    x_mms(0)

                    # --- encoder loop ---
                    for t in range(T2):
                        if t + 2 < T2:
                            xbt[t + 2] = dma_xb(t + 2)
                        if t > 0:
                            h_mms(t, hW)
                        # tail-fill work: next x-MMs, then the transpose block
                        # (grouped with cell's h-transposes to minimize PE
                        # matmul<->transpose mode switches)
                        if t + 1 < T2:
                            x_mms(t + 1)
                        cell(t, first=(t == 0))
                        if t + 2 < T2:
                            nc.tensor.transpose(gi[t % 2][0:64, 256:320],
                                                xbt.pop(t + 2), id64)
                            nc.scalar.copy(xsT[(t + 2) % 4][0:OC, :],
                                           gi[t % 2][0:64, 256:320])

                    _mark(nc, "dec_start")

                    def bias_mms(t):
                        # constant opener: beff = dec_b + head_b*dec_Wih, the
                        # folded-feedback bias (start=True on each region)
                        j = t % 2
                        for cg in range(2):
                            sl = slice(64 * cg, 64 * (cg + 1))
                            nc.tensor.matmul(gf[j][sl, 0:512], lhsT=onesB,
                                             rhs=beff[cg][0:1, 0:512],
                                             start=True, stop=False)
                            nc.tensor.matmul(gi[j][sl, 0:256], lhsT=onesB,
                                             rhs=beff[cg][0:1, 512:768],
                                             start=True, stop=False)
                            nc.tensor.matmul(go[j][sl, 0:256], lhsT=onesB,
                                             rhs=beff[cg][0:1, 768:1024],
                                             start=True, stop=False)

                    def head_mms(t):
                        # pred.T = head_w @ h.T + head_b (output only — the
                        # recurrence feedback is folded into dhW/beff)
                        hps = go[t % 2][0:1, 256:320]
                        for kc in range(4):
                            hsl, c0 = hTsl[kc]
                            nc.tensor.matmul(hps, lhsT=hdT[:, kc:kc + 1],
                                             rhs=hsl[:, c0:c0 + 64],
                                             start=(kc == 0), stop=False)
                        nc.tensor.matmul(hps, lhsT=hb, rhs=ones_row,
                                         start=False, stop=True)
                        predsb = xbp.tile([1, B], F32, tag="predsb", bufs=2)
                        nc.scalar.copy(predsb, hps)
                        return predsb

                    # step-0: z = dstart - pred(h_enc); z @ dec_Wih.T corrects
                    # the folded feedback exactly for the first input
                    bias_mms(0)
                    pred0 = head_mms(0)
                    nc.vector.tensor_sub(augz, dstart_row, pred0)
                    for cg in range(2):
                        sl = slice(64 * cg, 64 * (cg + 1))
                        nc.tensor.matmul(gf[0][sl, 0:512], lhsT=augz,
                                         rhs=dxW[cg][0:1, 0:512],
                                         start=False, stop=False)
                        nc.tensor.matmul(gi[0][sl, 0:256], lhsT=augz,
                                         rhs=dxW[cg][0:1, 512:768],
                                         start=False, stop=False)
                        nc.tensor.matmul(go[0][sl, 0:256], lhsT=augz,
                                         rhs=dxW[cg][0:1, 768:1024],
                                         start=False, stop=False)

                    # --- decoder loop ---
                    for t in range(OUT_STEPS):
                        h_mms(t, dhW)
                        if t + 1 < OUT_STEPS:
                            bias_mms(t + 1)
                        cell(t, first=False)
                        predsb = head_mms(t)
                        nc.tensor.transpose(gi[t % 2][0:64, 256:257], predsb,
                                            identity[:1, :1])
                        nc.scalar.copy(outF[:, t:t + 1], gi[t % 2][0:64, 256:257])

                    nc.sync.dma_start(out=out_d[:, :], in_=outF)

    _mark(nc, "end")
    nc.compile()
    return nc


_CACHED = {}


def kernel(**inputs):
    """Full-input entry: shard batch across 8 cores, run SPMD, gather."""
    from concourse.bass_utils import run_bass_kernel_spmd

    if "nc" not in _CACHED:
        _CACHED["nc"] = build_nc()
    nc = _CACHED["nc"]

    full = {k: np.ascontiguousarray(np.asarray(v, dtype=np.float32)) for k, v in inputs.items()}
    per_core = []
    for c in range(NCORES):
        sl = slice(c * B, (c + 1) * B)
        m = {}
        for k, v in full.items():
            if k in ("x", "decoder_start"):
                m[k] = np.ascontiguousarray(v[sl])
            else:
                m[k] = v
        per_core.append(m)

    res = run_bass_kernel_spmd(nc, per_core, core_ids=list(range(NCORES)))
    outs = [r["out"] for r in res.results]
    return np.concatenate(outs, axis=0)
